# revision 5
# baseline (speedup 1.0000x reference)
"""Discrete Hawkes conditional-intensity kernel for 8 Trainium2 NeuronCores.

Math
----
Reference computes, per query i with (t, s) = (t_i, s_i):

    lam_i = clip(mu[s] + alpha[s, s] * b * F[t, s], 1e-5)
    F[t, s] = sum_{tp < t} obs[tp, s] * exp(-b * (t - tp))

F obeys F[t+1] = e * (F[t] + obs[t]), e = exp(-b), i.e. it is an
exponentially-decayed prefix sum over time.  On device we build the full
table G[t, s] = mu[s] + alpha[s,s]*b*F[t, s] with a blocked formulation
(time blocks of 128 on the PE array + a 32-step cross-block carry), store
it to DRAM, then answer the queries with batched SWDGE row gathers
(dma_gather, 1024 descriptors each — the per-instruction SWDGE fixed cost
~1us makes per-element indirect DMAs 8x more expensive) followed by a
one-hot select of the right element from each 64-float row on the vector
engine.

Sharding: queries (t, s) are split 8x8192 across cores (data parallel);
obs / mu / alpha / beta are replicated.  No collectives needed.
"""

import os
import sys

import numpy as np

_REPO_CANDIDATES = ("/opt/trn_rl_repo", os.path.expanduser("~/.axon_site/_ro/trn_rl_repo"))
for _p in _REPO_CANDIDATES:
    if os.path.isdir(_p) and _p not in sys.path:
        sys.path.append(_p)

import concourse.bass as bass
import concourse.tile as tile
from concourse import bacc, mybir
from concourse.bass_utils import run_bass_kernel_spmd

# Problem constants (hardcoded per spec).
N_TIME = 4096
N_SPACE = 256
BATCH = 65536
N_CORES = 8
LAM_MIN = 1e-5

P = 128               # partitions / time-block size
J = N_TIME // P       # 32 time blocks
PER_CORE = BATCH // N_CORES   # 8192 queries per core
CH = 512              # matmul N-chunk (one PSUM bank)
NCH = (J * N_SPACE) // CH     # 16 chunks over the (j, s) flat axis

# Gather staging: the G table is viewed as 16384 rows x 64 f32 (256B rows,
# the dma_gather minimum).  Queries are routed host-side into 9 stages of
# 1024 (SWDGE descriptor-carveout limit per instruction); stage k only
# holds queries with t < 512*(k+1) so its gather can fire as soon as that
# part of G is in DRAM.  Slack: 9*1024 slots for 8192 queries.
NSTAGE = 9
SIDX = 1024           # queries per gather stage
SCOLS = SIDX // P     # 8 out columns per stage
FQ = NSTAGE * SCOLS   # 72 slot columns per partition
NSLOT = P * FQ        # 9216 slots per core
EROW = 64             # f32 elements per gathered row
NROWS = N_TIME * N_SPACE // EROW  # 16384

f32 = mybir.dt.float32
bf16 = mybir.dt.bfloat16
i32 = mybir.dt.int32
i16 = mybir.dt.int16
Alu = mybir.AluOpType
Act = mybir.ActivationFunctionType
Axis = mybir.AxisListType


def build_nc():
    nc = bacc.Bacc("TRN2", target_bir_lowering=False, debug=False)

    gidx_h = nc.dram_tensor("gidx", [P, NSTAGE * SIDX // 16], i16, kind="ExternalInput")
    emod_h = nc.dram_tensor("emod", [P, FQ], i32, kind="ExternalInput")
    obs_h = nc.dram_tensor("obs", [N_TIME, N_SPACE], i32, kind="ExternalInput")
    mu_h = nc.dram_tensor("mu", [N_SPACE], f32, kind="ExternalInput")
    alpha_h = nc.dram_tensor("alpha", [N_SPACE, N_SPACE], f32, kind="ExternalInput")
    beta_h = nc.dram_tensor("beta", [1], f32, kind="ExternalInput")
    g_h = nc.dram_tensor("gtab", [N_TIME * N_SPACE], f32, kind="Internal")
    out_h = nc.dram_tensor("out", [NSLOT], f32, kind="ExternalOutput")

    from contextlib import ExitStack

    with tile.TileContext(nc) as tc, ExitStack() as ctx:
        sb = ctx.enter_context(tc.tile_pool(name="sb", bufs=1))
        ps = ctx.enter_context(tc.tile_pool(name="ps", bufs=4, space="PSUM"))
        psr = ctx.enter_context(tc.tile_pool(name="psr", bufs=2, space="PSUM"))
        ps1 = ctx.enter_context(tc.tile_pool(name="ps1", bufs=1, space="PSUM"))
        sb2 = ctx.enter_context(tc.tile_pool(name="sb2", bufs=4))

        # ---- input loads -------------------------------------------------
        obs_view = obs_h.ap().rearrange("(j p) s -> p j s", p=P)
        obs_i = sb.tile([P, J, N_SPACE], i32)
        for q in range(8):
            nc.sync.dma_start(obs_i[:, 4 * q:4 * q + 4, :],
                              obs_view[:, 4 * q:4 * q + 4, :])

        beta_bc = sb.tile([P, 1], f32)
        nc.scalar.dma_start(beta_bc[:], bass.AP(beta_h, 0, [[0, P], [1, 1]]))

        adiag = sb.tile([1, N_SPACE], f32)
        nc.scalar.dma_start(adiag[:], bass.AP(alpha_h, 0, [[0, 1], [N_SPACE + 1, N_SPACE]]))

        rhs2 = sb.tile([2, J * N_SPACE], bf16)  # row0 = carry C flat, row1 = mu tiled
        mu_f = sb.tile([1, N_SPACE], f32)
        nc.scalar.dma_start(mu_f[:], bass.AP(mu_h, 0, [[0, 1], [1, N_SPACE]]))
        mu_b = sb.tile([1, N_SPACE], bf16)
        nc.vector.tensor_copy(mu_b[:], mu_f[:])
        nc.scalar.dma_start(
            rhs2[1:2, :].rearrange("o (j s) -> o j s", s=N_SPACE),
            mu_b[:].unsqueeze(1).broadcast_to((1, J, N_SPACE)))

        gidx = sb.tile([P, NSTAGE * SIDX // 16], i16)
        nc.scalar.dma_start(gidx[:], gidx_h.ap())
        emod = sb.tile([P, FQ], i32)
        nc.scalar.dma_start(emod[:], emod_h.ap())

        # ---- runtime constants from beta --------------------------------
        negb = sb.tile([P, 1], f32)
        nc.vector.tensor_scalar(out=negb[:], in0=beta_bc[:], scalar1=-1.0,
                                scalar2=None, op0=Alu.mult)
        negb128 = sb.tile([P, 1], f32)
        nc.vector.tensor_scalar(out=negb128[:], in0=negb[:], scalar1=128.0,
                                scalar2=None, op0=Alu.mult)

        # LdT[tp, m] = exp(-b (m - tp)) for tp < m else 0   (within-block decay)
        xd = sb.tile([P, P], i32)
        nc.gpsimd.iota(xd[:], [[1, P]], base=0, channel_multiplier=-1)   # f - p
        lda = sb.tile([P, P], f32)
        nc.vector.tensor_scalar(out=lda[:], in0=xd[:], scalar1=negb[:],
                                scalar2=None, op0=Alu.mult)
        ldb = sb.tile([P, P], f32)
        nc.vector.tensor_scalar(out=ldb[:], in0=xd[:], scalar1=1000.0,
                                scalar2=-1000.0, op0=Alu.mult, op1=Alu.add)
        ldm = sb.tile([P, P], f32)
        nc.vector.tensor_tensor(out=ldm[:], in0=lda[:], in1=ldb[:], op=Alu.min)
        ldt = sb.tile([P, P], f32)
        nc.scalar.activation(ldt[:], ldm[:], Act.Exp)
        ldtb = sb.tile([P, P], bf16)
        nc.vector.tensor_copy(ldtb[:], ldt[:])

        # v[tp] = exp(-b (128 - tp))  (end-of-block carry weights)
        xv = sb.tile([P, 1], i32)
        nc.gpsimd.iota(xv[:], [[0, 1]], base=P, channel_multiplier=-1)   # 128 - p
        vm = sb.tile([P, 1], f32)
        nc.vector.tensor_scalar(out=vm[:], in0=xv[:], scalar1=negb[:],
                                scalar2=None, op0=Alu.mult)
        vv = sb.tile([P, 1], f32)
        nc.scalar.activation(vv[:], vm[:], Act.Exp)
        vvb = sb.tile([P, 1], bf16)
        nc.vector.tensor_copy(vvb[:], vv[:])

        # LcT[k, j] = exp(-128 b (j - 1 - k)) for k <= j-1 else 0  (carry matrix)
        xc = sb.tile([J, J], i32)
        nc.gpsimd.iota(xc[:], [[1, J]], base=-1, channel_multiplier=-1)  # f - 1 - p
        lca = sb.tile([J, J], f32)
        nc.vector.tensor_scalar(out=lca[:], in0=xc[:], scalar1=negb128[:J, :],
                                scalar2=None, op0=Alu.mult)
        lcb = sb.tile([J, J], f32)
        nc.vector.tensor_scalar(out=lcb[:], in0=xc[:], scalar1=1000.0,
                                scalar2=None, op0=Alu.mult)
        lcm = sb.tile([J, J], f32)
        nc.vector.tensor_tensor(out=lcm[:], in0=lca[:], in1=lcb[:], op=Alu.min)
        lct = sb.tile([J, J], f32)
        nc.scalar.activation(lct[:], lcm[:], Act.Exp)

        # u2: row0 = u_i = exp(-b i), row1 = ones (mu term).
        # scale vector [-b; 0] makes exp produce both rows at once.
        negb01 = sb.tile([2, 1], f32)
        nc.vector.memset(negb01[:], 0.0)
        nc.vector.tensor_copy(negb01[0:1, :], negb[0:1, :])
        xu = sb.tile([2, P], i32)
        nc.gpsimd.iota(xu[:], [[1, P]], base=0, channel_multiplier=0)    # f
        um = sb.tile([2, P], f32)
        nc.vector.tensor_scalar(out=um[:], in0=xu[:], scalar1=negb01[:],
                                scalar2=None, op0=Alu.mult)
        u2 = sb.tile([2, P], f32)
        nc.scalar.activation(u2[:], um[:], Act.Exp)
        u2b = sb.tile([2, P], bf16)
        nc.vector.tensor_copy(u2b[:], u2[:])

        # asb[s] = b * alpha[s, s], broadcast to all 128 partitions via PE
        asb_row = sb.tile([1, N_SPACE], f32)
        nc.vector.tensor_scalar(out=asb_row[:], in0=adiag[:],
                                scalar1=beta_bc[:1, :], scalar2=None, op0=Alu.mult)
        ones1 = sb.tile([1, P], f32)
        nc.vector.memset(ones1[:], 1.0)
        asb_ps = ps1.tile([P, N_SPACE], f32)
        nc.tensor.matmul(asb_ps[:], lhsT=ones1[:], rhs=asb_row[:], start=True, stop=True)
        asb_bc = sb.tile([P, N_SPACE], f32)
        nc.vector.tensor_copy(asb_bc[:], asb_ps[:])

        # iota64[p, e] = e  (for the one-hot row-element select)
        iota64 = sb.tile([P, EROW], i32)
        nc.gpsimd.iota(iota64[:], [[1, EROW]], base=0, channel_multiplier=0)

        # obs_f[tp, j, s] = obs * asb[s]   (convert + scale, 4 chunked DVE passes)
        obs_f = sb.tile([P, J * N_SPACE], bf16)
        obs_ff = obs_f[:]                # [P, 8192] flat view
        obs_f3 = obs_f[:].rearrange("p (j s) -> p j s", s=N_SPACE)
        for q in range(4):
            nc.vector.tensor_tensor(
                out=obs_f3[:, 8 * q:8 * q + 8, :],
                in0=obs_i[:, 8 * q:8 * q + 8, :],
                in1=asb_bc[:].unsqueeze(1).broadcast_to((P, 8, N_SPACE)),
                op=Alu.mult,
            )

        # ---- fused build + staged gather pipeline -----------------------
        # Quarter k: reduce r over its 4 obs chunks, extend the carry,
        # build + store its 4 G chunks, then fire the row-gather stages
        # whose t-range is now fully in DRAM.
        r_flat = sb.tile([1, J * N_SPACE], f32)
        r32 = sb.tile([J, N_SPACE], f32)
        rhs2_j = rhs2[0:1, :].rearrange("o (j s) -> o j s", s=N_SPACE)
        g_store = bass.AP(g_h, 0, [[N_SPACE, P], [P * N_SPACE, J], [1, N_SPACE]])

        gout = sb.tile([P, FQ, EROW], f32)

        def fire_stage(st):
            bound = min(2048 * (st + 1), NROWS)
            nc.gpsimd.dma_gather(
                out_ap=gout[:, SCOLS * st:SCOLS * (st + 1), :],
                in_ap=bass.AP(g_h, 0, [[EROW, bound], [1, EROW]]),
                idxs_ap=gidx[:, 64 * st:64 * (st + 1)],
                num_idxs=SIDX, num_idxs_reg=SIDX, elem_size=EROW)
            mask = sb2.tile([P, SCOLS, EROW], f32, tag="mask")
            nc.vector.tensor_tensor(
                out=mask[:],
                in0=iota64[:].unsqueeze(1).broadcast_to((P, SCOLS, EROW)),
                in1=emod[:, SCOLS * st:SCOLS * (st + 1)].unsqueeze(2)
                    .broadcast_to((P, SCOLS, EROW)),
                op=Alu.is_equal)
            prod = sb2.tile([P, SCOLS, EROW], f32, tag="prod")
            nc.vector.tensor_tensor(
                out=prod[:], in0=gout[:, SCOLS * st:SCOLS * (st + 1), :],
                in1=mask[:], op=Alu.mult)
            val = sb2.tile([P, SCOLS], f32, tag="val")
            nc.vector.tensor_reduce(out=val[:], in_=prod[:], axis=Axis.X,
                                    op=Alu.add)
            lam = sb2.tile([P, SCOLS], f32, tag="lam")
            nc.vector.tensor_scalar(out=lam[:], in0=val[:],
                                    scalar1=float(LAM_MIN), scalar2=None,
                                    op0=Alu.max)
            nc.scalar.dma_start(
                bass.AP(out_h, SCOLS * st, [[FQ, P], [1, SCOLS]]), lam[:])

        for k in range(4):
            for c in range(4 * k, 4 * k + 4):
                r_ps = psr.tile([1, CH], f32)
                nc.tensor.matmul(r_ps[:], lhsT=vvb[:],
                                 rhs=obs_ff[:, c * CH:(c + 1) * CH],
                                 start=True, stop=True)
                nc.scalar.activation(r_flat[:, c * CH:(c + 1) * CH], r_ps[:],
                                     Act.Copy)
            nc.sync.dma_start(r32[8 * k:8 * k + 8, :],
                              r_flat[:, 2048 * k:2048 * (k + 1)])
            c_ps = ps1.tile([8, N_SPACE], f32, tag="cps")
            nc.tensor.matmul(c_ps[:], lhsT=lct[0:8 * (k + 1), 8 * k:8 * (k + 1)],
                             rhs=r32[0:8 * (k + 1), :], start=True, stop=True)
            c32 = sb2.tile([8, N_SPACE], bf16, tag="c32")
            nc.vector.tensor_copy(c32[:], c_ps[:])
            nc.sync.dma_start(rhs2_j[:, 8 * k:8 * k + 8, :], c32[:])

            for c in range(4 * k, 4 * k + 4):
                pch = ps.tile([P, CH], f32)
                nc.tensor.matmul(pch[:], lhsT=ldtb[:],
                                 rhs=obs_ff[:, c * CH:(c + 1) * CH],
                                 start=True, stop=True)
                nc.tensor.matmul(pch[:], lhsT=u2b[:],
                                 rhs=rhs2[:, c * CH:(c + 1) * CH],
                                 start=False, stop=True, skip_group_check=True)
                gch = sb2.tile([P, CH], f32, tag="gch")
                if c % 2 == 0:
                    nc.vector.tensor_copy(gch[:], pch[:])
                else:
                    nc.scalar.activation(gch[:], pch[:], Act.Copy)
                jj = c * CH // N_SPACE
                eng = nc.sync if c % 2 == 0 else nc.scalar
                eng.dma_start(g_store[:, jj:jj + CH // N_SPACE, :], gch[:])

            # stages whose t-bound is now stored: t < 512(st+1) needs chunks
            # <= 2st+1; after quarter k chunks <= 4k+3 are stored.
            for st in range(2 * k, 2 * k + 2):
                fire_stage(st)
        fire_stage(8)

    nc.compile()
    return nc


_NC_CACHE = None


def _get_nc():
    global _NC_CACHE
    if _NC_CACHE is None:
        _NC_CACHE = build_nc()
    return _NC_CACHE


def _route_queries(tc_, sc_):
    """Route one core's queries into gather-stage slots.

    Stage k (capacity 1024) may only hold queries with t < 512*(k+1)
    (stages 7 and 8 hold anything).  Query q of stage k lands in device
    slot (p, col) = (q % 128, 8k + q // 128); its gathered row is
    t*4 + (s >> 6) and its element-within-row is s & 63.

    Returns (gidx [128, 576] int16, emod [128, 72] int32,
    (dev_pos, orig_pos)) with out[orig_pos] = dev_out[dev_pos] on the
    [NSLOT] flat device output (flat = p * FQ + col).
    """
    n = tc_.shape[0]
    order = np.argsort(tc_, kind="stable")
    ts = tc_[order]
    rows_all = tc_.astype(np.int64) * 4 + (sc_.astype(np.int64) >> 6)
    emods_all = sc_.astype(np.int64) & 63

    gidx = np.zeros((P, NSTAGE * SIDX // 16), np.int16)
    emod = np.zeros((P, FQ), np.int32)
    dev_parts, orig_parts = [], []
    lo = 0
    for k in range(NSTAGE):
        bound = min(512 * (k + 1), N_TIME)
        hi = np.searchsorted(ts, bound, side="left")
        take = min(SIDX, hi - lo) if k < NSTAGE - 1 else (n - lo)
        if take > SIDX:
            raise RuntimeError("query t-distribution infeasible for stages")
        sel = order[lo:lo + take]
        q = np.arange(take)
        p = q % P
        cl = q // P
        # wrapped idx layout: query q of the stage sits at partition q%16
        # (replicated across the 8 groups), free slot 64k + q//16.
        rows = rows_all[sel].astype(np.int16)
        for g in range(8):
            gidx[16 * g + (q % 16), 64 * k + q // 16] = rows
        emod[p, SCOLS * k + cl] = emods_all[sel]
        dev_parts.append(p * FQ + SCOLS * k + cl)
        orig_parts.append(sel)
        lo += take
    dev_pos = np.concatenate(dev_parts)
    orig_pos = np.concatenate(orig_parts)
    return gidx, emod, (dev_pos, orig_pos)


def _make_in_maps(t, s, obs, mu, alpha, beta):
    in_maps, perms = [], []
    for c in range(N_CORES):
        sl = slice(c * PER_CORE, (c + 1) * PER_CORE)
        gidx, emod, perm = _route_queries(t[sl], s[sl])
        perms.append(perm)
        in_maps.append({
            "gidx": gidx, "emod": emod,
            "obs": obs, "mu": mu, "alpha": alpha, "beta": beta,
        })
    return in_maps, perms


def kernel(t, s, obs, mu, alpha, beta, **_unused):
    t = np.ascontiguousarray(np.asarray(t, dtype=np.int32))
    s = np.ascontiguousarray(np.asarray(s, dtype=np.int32))
    obs = np.ascontiguousarray(np.asarray(obs, dtype=np.int32))
    mu = np.ascontiguousarray(np.asarray(mu, dtype=np.float32))
    alpha = np.ascontiguousarray(np.asarray(alpha, dtype=np.float32))
    beta = np.ascontiguousarray(np.asarray(beta, dtype=np.float32))

    nc = _get_nc()
    in_maps, perms = _make_in_maps(t, s, obs, mu, alpha, beta)
    res = run_bass_kernel_spmd(nc, in_maps, core_ids=list(range(N_CORES)))
    outs = []
    for c in range(N_CORES):
        dev = res.results[c]["out"]          # [NSLOT]
        o = np.empty(PER_CORE, np.float32)
        o[perms[c][1]] = dev[perms[c][0]]
        outs.append(o)
    return np.concatenate(outs).astype(np.float32)


if __name__ == "__main__":
    # quick self-check against a numpy re-implementation on random data
    rng = np.random.default_rng(0)
    t = rng.integers(0, N_TIME, BATCH).astype(np.int32)
    s = rng.integers(0, N_SPACE, BATCH).astype(np.int32)
    obs = rng.integers(0, 10, (N_TIME, N_SPACE)).astype(np.int32)
    mu = rng.random(N_SPACE, dtype=np.float32)
    alpha = rng.random((N_SPACE, N_SPACE), dtype=np.float32)
    beta = (rng.random(1, dtype=np.float32) + 0.1).astype(np.float32)

    got = kernel(t=t, s=s, obs=obs, mu=mu, alpha=alpha, beta=beta)

    b = float(beta[0])
    e = np.exp(-b)
    F = np.zeros((N_TIME, N_SPACE), np.float64)
    for tt in range(1, N_TIME):
        F[tt] = e * (F[tt - 1] + obs[tt - 1])
    G = np.clip(mu[None, :] + np.diag(alpha)[None, :] * b * F, LAM_MIN, None)
    want = G[t, s].astype(np.float32)
    err = np.abs(got - want) / np.maximum(np.abs(want), 1e-6)
    print("max rel err:", err.max(), "mean:", err.mean())


# revision 6
# speedup vs baseline: 1.0177x; 1.0177x over previous
"""Discrete Hawkes conditional-intensity kernel for 8 Trainium2 NeuronCores.

Math
----
Reference computes, per query i with (t, s) = (t_i, s_i):

    lam_i = clip(mu[s] + alpha[s, s] * b * F[t, s], 1e-5)
    F[t, s] = sum_{tp < t} obs[tp, s] * exp(-b * (t - tp))

F obeys F[t+1] = e * (F[t] + obs[t]), e = exp(-b), i.e. it is an
exponentially-decayed prefix sum over time.  On device we build the full
table G[t, s] = mu[s] + alpha[s,s]*b*F[t, s] with a blocked formulation
(time blocks of 128 on the PE array + a 32-step cross-block carry), store
it to DRAM, then answer the 8192 queries per core with per-partition
indirect-DMA element gathers.

The gather is the bottleneck: SWDGE descriptor generation on GPSIMD costs
~1us fixed per indirect DMA (128 descriptors, one per partition), and no
HW path batches data-dependent descriptors faster (dma_gather ucode runs
~7ns/descriptor; ap_gather ~27ns/index).  So the 64 column gathers are
~70us of serial GPSIMD time no matter what; the schedule hides nearly all
of it behind the table build by routing queries host-side into 18 stages
keyed to the 256-timestep store chunks, so gathering starts as soon as
the first chunk of G is in DRAM.

Sharding: queries (t, s) are split 8x8192 across cores (data parallel);
obs / mu / alpha / beta are replicated.  No collectives needed.
"""

import os
import sys

import numpy as np

_REPO_CANDIDATES = ("/opt/trn_rl_repo", os.path.expanduser("~/.axon_site/_ro/trn_rl_repo"))
for _p in _REPO_CANDIDATES:
    if os.path.isdir(_p) and _p not in sys.path:
        sys.path.append(_p)

import concourse.bass as bass
import concourse.tile as tile
from concourse import bacc, mybir
from concourse.bass_utils import run_bass_kernel_spmd

# Problem constants (hardcoded per spec).
N_TIME = 4096
N_SPACE = 256
BATCH = 65536
N_CORES = 8
LAM_MIN = 1e-5

P = 128               # partitions / time-block size
J = N_TIME // P       # 32 time blocks
PER_CORE = BATCH // N_CORES   # 8192 queries per core
CH = 512              # matmul N-chunk (one PSUM bank)
NCH = (J * N_SPACE) // CH     # 16 chunks over the (j, s) flat axis

# Gather slot layout: 18 stages of 4 columns (128 queries each).  Stage c
# (c < 16) only holds queries with t < 256*(c+1), i.e. covered by G store
# chunks 0..c, so its gathers can fire right after chunk c is stored.
# Stages 16-17 hold the spill (any t); 9216 slots for 8192 queries.
NSTAGE = 18
SCOLS = 4
FQ = NSTAGE * SCOLS   # 72 query slot columns per partition
NSLOT = P * FQ        # 9216 slots per core

f32 = mybir.dt.float32
bf16 = mybir.dt.bfloat16
i32 = mybir.dt.int32
Alu = mybir.AluOpType
Act = mybir.ActivationFunctionType


def build_nc():
    nc = bacc.Bacc("TRN2", target_bir_lowering=False, debug=False,
                   dynamic_dma_scratch_size=65536)

    gidx_h = nc.dram_tensor("gidx", [NSLOT], i32, kind="ExternalInput")
    obs_h = nc.dram_tensor("obs", [N_TIME, N_SPACE], i32, kind="ExternalInput")
    mu_h = nc.dram_tensor("mu", [N_SPACE], f32, kind="ExternalInput")
    alpha_h = nc.dram_tensor("alpha", [N_SPACE, N_SPACE], f32, kind="ExternalInput")
    beta_h = nc.dram_tensor("beta", [1], f32, kind="ExternalInput")
    g_h = nc.dram_tensor("gtab", [N_TIME * N_SPACE + 2], f32, kind="Internal")
    out_h = nc.dram_tensor("out", [NSLOT], f32, kind="ExternalOutput")

    from contextlib import ExitStack

    with tile.TileContext(nc) as tc, ExitStack() as ctx:
        sb = ctx.enter_context(tc.tile_pool(name="sb", bufs=1))
        ps = ctx.enter_context(tc.tile_pool(name="ps", bufs=4, space="PSUM"))
        psr = ctx.enter_context(tc.tile_pool(name="psr", bufs=2, space="PSUM"))
        ps1 = ctx.enter_context(tc.tile_pool(name="ps1", bufs=1, space="PSUM"))
        sb2 = ctx.enter_context(tc.tile_pool(name="sb2", bufs=4))

        # ---- input loads -------------------------------------------------
        obs_view = obs_h.ap().rearrange("(j p) s -> p j s", p=P)
        obs_i = sb.tile([P, J, N_SPACE], i32)
        for q in range(8):
            nc.sync.dma_start(obs_i[:, 4 * q:4 * q + 4, :],
                              obs_view[:, 4 * q:4 * q + 4, :])

        beta_bc = sb.tile([P, 1], f32)
        nc.scalar.dma_start(beta_bc[:], bass.AP(beta_h, 0, [[0, P], [1, 1]]))

        adiag = sb.tile([1, N_SPACE], f32)
        nc.scalar.dma_start(adiag[:], bass.AP(alpha_h, 0, [[0, 1], [N_SPACE + 1, N_SPACE]]))

        rhs2 = sb.tile([2, J * N_SPACE], bf16)  # row0 = carry C flat, row1 = mu tiled
        mu_f = sb.tile([1, N_SPACE], f32)
        nc.scalar.dma_start(mu_f[:], bass.AP(mu_h, 0, [[0, 1], [1, N_SPACE]]))
        mu_b = sb.tile([1, N_SPACE], bf16)
        nc.vector.tensor_copy(mu_b[:], mu_f[:])
        nc.scalar.dma_start(
            rhs2[1:2, :].rearrange("o (j s) -> o j s", s=N_SPACE),
            mu_b[:].unsqueeze(1).broadcast_to((1, J, N_SPACE)))

        idx = sb.tile([P, FQ], i32)
        nc.scalar.dma_start(idx[:], bass.AP(gidx_h, 0, [[FQ, P], [1, FQ]]))

        # ---- runtime constants from beta --------------------------------
        negb = sb.tile([P, 1], f32)
        nc.vector.tensor_scalar(out=negb[:], in0=beta_bc[:], scalar1=-1.0,
                                scalar2=None, op0=Alu.mult)
        negb128 = sb.tile([P, 1], f32)
        nc.vector.tensor_scalar(out=negb128[:], in0=negb[:], scalar1=128.0,
                                scalar2=None, op0=Alu.mult)

        # LdT[tp, m] = exp(-b (m - tp)) for tp < m else 0   (within-block decay)
        xd = sb.tile([P, P], i32)
        nc.gpsimd.iota(xd[:], [[1, P]], base=0, channel_multiplier=-1)   # f - p
        lda = sb.tile([P, P], f32)
        nc.vector.tensor_scalar(out=lda[:], in0=xd[:], scalar1=negb[:],
                                scalar2=None, op0=Alu.mult)
        ldb = sb.tile([P, P], f32)
        nc.vector.tensor_scalar(out=ldb[:], in0=xd[:], scalar1=1000.0,
                                scalar2=-1000.0, op0=Alu.mult, op1=Alu.add)
        ldm = sb.tile([P, P], f32)
        nc.vector.tensor_tensor(out=ldm[:], in0=lda[:], in1=ldb[:], op=Alu.min)
        ldt = sb.tile([P, P], f32)
        nc.scalar.activation(ldt[:], ldm[:], Act.Exp)
        ldtb = sb.tile([P, P], bf16)
        nc.vector.tensor_copy(ldtb[:], ldt[:])

        # v[tp] = exp(-b (128 - tp))  (end-of-block carry weights)
        xv = sb.tile([P, 1], i32)
        nc.gpsimd.iota(xv[:], [[0, 1]], base=P, channel_multiplier=-1)   # 128 - p
        vm = sb.tile([P, 1], f32)
        nc.vector.tensor_scalar(out=vm[:], in0=xv[:], scalar1=negb[:],
                                scalar2=None, op0=Alu.mult)
        vv = sb.tile([P, 1], f32)
        nc.scalar.activation(vv[:], vm[:], Act.Exp)
        vvb = sb.tile([P, 1], bf16)
        nc.vector.tensor_copy(vvb[:], vv[:])

        # LcT[k, j] = exp(-128 b (j - 1 - k)) for k <= j-1 else 0  (carry matrix)
        xc = sb.tile([J, J], i32)
        nc.gpsimd.iota(xc[:], [[1, J]], base=-1, channel_multiplier=-1)  # f - 1 - p
        lca = sb.tile([J, J], f32)
        nc.vector.tensor_scalar(out=lca[:], in0=xc[:], scalar1=negb128[:J, :],
                                scalar2=None, op0=Alu.mult)
        lcb = sb.tile([J, J], f32)
        nc.vector.tensor_scalar(out=lcb[:], in0=xc[:], scalar1=1000.0,
                                scalar2=None, op0=Alu.mult)
        lcm = sb.tile([J, J], f32)
        nc.vector.tensor_tensor(out=lcm[:], in0=lca[:], in1=lcb[:], op=Alu.min)
        lct = sb.tile([J, J], f32)
        nc.scalar.activation(lct[:], lcm[:], Act.Exp)

        # u2: row0 = u_i = exp(-b i), row1 = ones (mu term).
        # scale vector [-b; 0] makes exp produce both rows at once.
        negb01 = sb.tile([2, 1], f32)
        nc.vector.memset(negb01[:], 0.0)
        nc.vector.tensor_copy(negb01[0:1, :], negb[0:1, :])
        xu = sb.tile([2, P], i32)
        nc.gpsimd.iota(xu[:], [[1, P]], base=0, channel_multiplier=0)    # f
        um = sb.tile([2, P], f32)
        nc.vector.tensor_scalar(out=um[:], in0=xu[:], scalar1=negb01[:],
                                scalar2=None, op0=Alu.mult)
        u2 = sb.tile([2, P], f32)
        nc.scalar.activation(u2[:], um[:], Act.Exp)
        u2b = sb.tile([2, P], bf16)
        nc.vector.tensor_copy(u2b[:], u2[:])

        # asb[s] = b * alpha[s, s], broadcast to all 128 partitions via PE
        asb_row = sb.tile([1, N_SPACE], f32)
        nc.vector.tensor_scalar(out=asb_row[:], in0=adiag[:],
                                scalar1=beta_bc[:1, :], scalar2=None, op0=Alu.mult)
        ones1 = sb.tile([1, P], f32)
        nc.vector.memset(ones1[:], 1.0)
        asb_ps = ps1.tile([P, N_SPACE], f32)
        nc.tensor.matmul(asb_ps[:], lhsT=ones1[:], rhs=asb_row[:], start=True, stop=True)
        asb_bc = sb.tile([P, N_SPACE], f32)
        nc.vector.tensor_copy(asb_bc[:], asb_ps[:])

        # obs_f[tp, j, s] = obs * asb[s]   (convert + scale, 4 chunked DVE passes)
        obs_f = sb.tile([P, J * N_SPACE], bf16)
        obs_ff = obs_f[:]                # [P, 8192] flat view
        obs_f3 = obs_f[:].rearrange("p (j s) -> p j s", s=N_SPACE)
        for q in range(4):
            nc.vector.tensor_tensor(
                out=obs_f3[:, 8 * q:8 * q + 8, :],
                in0=obs_i[:, 8 * q:8 * q + 8, :],
                in1=asb_bc[:].unsqueeze(1).broadcast_to((P, 8, N_SPACE)),
                op=Alu.mult,
            )

        # ---- fused build + per-chunk gather pipeline --------------------
        r_flat = sb.tile([1, J * N_SPACE], f32)
        r32 = sb.tile([J, N_SPACE], f32)
        rhs2_j = rhs2[0:1, :].rearrange("o (j s) -> o j s", s=N_SPACE)
        g_store = bass.AP(g_h, 0, [[N_SPACE, P], [P * N_SPACE, J], [1, N_SPACE]])

        gath = sb.tile([P, 2 * FQ], f32)
        views = [bass.AP(g_h, 0, [[1, min(256 * (c + 1), N_TIME) * N_SPACE], [1, 1]])
                 for c in range(NSTAGE - 1)]
        views.append(bass.AP(g_h, 0, [[1, N_TIME * N_SPACE + 2], [1, 1]]))
        zpad = sb.tile([1, 2], f32)
        nc.vector.memset(zpad[:], 0.0)
        nc.sync.dma_start(bass.AP(g_h, N_TIME * N_SPACE, [[1, 1], [1, 2]]), zpad[:])

        def fire_stage(st):
            for f in range(SCOLS * st, SCOLS * (st + 1)):
                nc.gpsimd.indirect_dma_start(
                    out=gath[:, 2 * f:2 * f + 2],
                    out_offset=None,
                    in_=views[st],
                    in_offset=bass.IndirectOffsetOnAxis(ap=idx[:, f:f + 1],
                                                        axis=0),
                )
            lam = sb2.tile([P, SCOLS], f32, tag="lam")
            nc.vector.tensor_scalar(
                out=lam[:].rearrange("p (f o) -> p f o", o=1),
                in0=gath[:].rearrange("p (f o) -> p f o", o=2)[
                    :, SCOLS * st:SCOLS * (st + 1), 0:1],
                scalar1=float(LAM_MIN), scalar2=None, op0=Alu.max)
            nc.scalar.dma_start(
                bass.AP(out_h, SCOLS * st, [[FQ, P], [1, SCOLS]]), lam[:])

        for k in range(4):
            for c in range(4 * k, 4 * k + 4):
                r_ps = psr.tile([1, CH], f32)
                nc.tensor.matmul(r_ps[:], lhsT=vvb[:],
                                 rhs=obs_ff[:, c * CH:(c + 1) * CH],
                                 start=True, stop=True)
                nc.scalar.activation(r_flat[:, c * CH:(c + 1) * CH], r_ps[:],
                                     Act.Copy)
            nc.sync.dma_start(r32[8 * k:8 * k + 8, :],
                              r_flat[:, 2048 * k:2048 * (k + 1)])
            c_ps = ps1.tile([8, N_SPACE], f32, tag="cps")
            nc.tensor.matmul(c_ps[:], lhsT=lct[0:8 * (k + 1), 8 * k:8 * (k + 1)],
                             rhs=r32[0:8 * (k + 1), :], start=True, stop=True)
            c32 = sb2.tile([8, N_SPACE], bf16, tag="c32")
            nc.vector.tensor_copy(c32[:], c_ps[:])
            nc.sync.dma_start(rhs2_j[:, 8 * k:8 * k + 8, :], c32[:])

            for c in range(4 * k, 4 * k + 4):
                pch = ps.tile([P, CH], f32)
                nc.tensor.matmul(pch[:], lhsT=ldtb[:],
                                 rhs=obs_ff[:, c * CH:(c + 1) * CH],
                                 start=True, stop=True)
                nc.tensor.matmul(pch[:], lhsT=u2b[:],
                                 rhs=rhs2[:, c * CH:(c + 1) * CH],
                                 start=False, stop=True, skip_group_check=True)
                gch = sb2.tile([P, CH], f32, tag="gch")
                if c % 2 == 0:
                    nc.vector.tensor_copy(gch[:], pch[:])
                else:
                    nc.scalar.activation(gch[:], pch[:], Act.Copy)
                jj = c * CH // N_SPACE
                eng = nc.sync if c % 2 == 0 else nc.scalar
                eng.dma_start(g_store[:, jj:jj + CH // N_SPACE, :], gch[:])
                # chunk c covers t in [256c, 256c+256): stage c unlocked.
                fire_stage(c)
        fire_stage(16)
        fire_stage(17)

    nc.compile()
    return nc


_NC_CACHE = None


def _get_nc():
    global _NC_CACHE
    if _NC_CACHE is None:
        _NC_CACHE = build_nc()
    return _NC_CACHE


def _route_queries(tc_, sc_):
    """Route one core's queries into gather-stage slots.

    Stage c < 16 (capacity 512 = 4 columns x 128 partitions) may only hold
    queries with t < 256*(c+1); stages 16-17 hold anything.  Query q of
    stage c lands in device slot (p, col) = (q % 128, 4c + q // 128).

    Returns (gidx [NSLOT] int32 flat t*256+s per slot, (dev_pos, orig_pos))
    with out[orig_pos] = dev_out[dev_pos] on the [NSLOT] flat device
    output (flat = p * FQ + col).
    """
    n = tc_.shape[0]
    order = np.argsort(tc_, kind="stable")
    ts = tc_[order]
    flat_all = tc_.astype(np.int64) * N_SPACE + sc_.astype(np.int64)

    gidx = np.zeros((P, FQ), np.int32)
    dev_parts, orig_parts = [], []
    lo = 0
    cap = P * SCOLS
    for c in range(NSTAGE):
        bound = min(256 * (c + 1), N_TIME)
        hi = np.searchsorted(ts, bound, side="left")
        take = min(cap, hi - lo) if c < NSTAGE - 1 else (n - lo)
        if take > cap:
            raise RuntimeError("query t-distribution infeasible for stages")
        sel = order[lo:lo + take]
        q = np.arange(take)
        p = q % P
        col = SCOLS * c + q // P
        gidx[p, col] = flat_all[sel]
        dev_parts.append(p * FQ + col)
        orig_parts.append(sel)
        lo += take
    dev_pos = np.concatenate(dev_parts)
    orig_pos = np.concatenate(orig_parts)
    return gidx.reshape(-1), (dev_pos, orig_pos)


def _make_in_maps(t, s, obs, mu, alpha, beta):
    in_maps, perms = [], []
    for c in range(N_CORES):
        sl = slice(c * PER_CORE, (c + 1) * PER_CORE)
        gidx, perm = _route_queries(t[sl], s[sl])
        perms.append(perm)
        in_maps.append({
            "gidx": gidx,
            "obs": obs, "mu": mu, "alpha": alpha, "beta": beta,
        })
    return in_maps, perms


def kernel(t, s, obs, mu, alpha, beta, **_unused):
    t = np.ascontiguousarray(np.asarray(t, dtype=np.int32))
    s = np.ascontiguousarray(np.asarray(s, dtype=np.int32))
    obs = np.ascontiguousarray(np.asarray(obs, dtype=np.int32))
    mu = np.ascontiguousarray(np.asarray(mu, dtype=np.float32))
    alpha = np.ascontiguousarray(np.asarray(alpha, dtype=np.float32))
    beta = np.ascontiguousarray(np.asarray(beta, dtype=np.float32))

    nc = _get_nc()
    in_maps, perms = _make_in_maps(t, s, obs, mu, alpha, beta)
    res = run_bass_kernel_spmd(nc, in_maps, core_ids=list(range(N_CORES)))
    outs = []
    for c in range(N_CORES):
        dev = res.results[c]["out"]          # [NSLOT]
        o = np.empty(PER_CORE, np.float32)
        o[perms[c][1]] = dev[perms[c][0]]
        outs.append(o)
    return np.concatenate(outs).astype(np.float32)


if __name__ == "__main__":
    # quick self-check against a numpy re-implementation on random data
    rng = np.random.default_rng(0)
    t = rng.integers(0, N_TIME, BATCH).astype(np.int32)
    s = rng.integers(0, N_SPACE, BATCH).astype(np.int32)
    obs = rng.integers(0, 10, (N_TIME, N_SPACE)).astype(np.int32)
    mu = rng.random(N_SPACE, dtype=np.float32)
    alpha = rng.random((N_SPACE, N_SPACE), dtype=np.float32)
    beta = (rng.random(1, dtype=np.float32) + 0.1).astype(np.float32)

    got = kernel(t=t, s=s, obs=obs, mu=mu, alpha=alpha, beta=beta)

    b = float(beta[0])
    e = np.exp(-b)
    F = np.zeros((N_TIME, N_SPACE), np.float64)
    for tt in range(1, N_TIME):
        F[tt] = e * (F[tt - 1] + obs[tt - 1])
    G = np.clip(mu[None, :] + np.diag(alpha)[None, :] * b * F, LAM_MIN, None)
    want = G[t, s].astype(np.float32)
    err = np.abs(got - want) / np.maximum(np.abs(want), 1e-6)
    print("max rel err:", err.max(), "mean:", err.mean())


# revision 18
# speedup vs baseline: 1.0780x; 1.0593x over previous
"""Discrete Hawkes conditional-intensity kernel for 8 Trainium2 NeuronCores.

Math
----
Reference computes, per query i with (t, s) = (t_i, s_i):

    lam_i = clip(mu[s] + alpha[s, s] * b * F[t, s], 1e-5)
    F[t, s] = sum_{tp < t} obs[tp, s] * exp(-b * (t - tp))

F obeys F[t+1] = e * (F[t] + obs[t]), e = exp(-b), i.e. it is an
exponentially-decayed prefix sum over time.  On device we build the full
table G[t, s] = mu[s] + alpha[s,s]*b*F[t, s] with a blocked formulation
(time blocks of 128 on the PE array + a 32-step cross-block carry), store
it to DRAM, then answer the 8192 queries per core with per-partition
indirect-DMA element gathers.

The gather is the bottleneck: SWDGE descriptor generation on GPSIMD costs
~1us fixed per indirect DMA (128 descriptors, one per partition), and no
HW path batches data-dependent descriptors faster (dma_gather ucode runs
~7ns/descriptor; ap_gather ~27ns/index).  So the 64 column gathers are
~70us of serial GPSIMD time no matter what; the schedule hides nearly all
of it behind the table build by routing queries host-side into 18 stages
keyed to the 256-timestep store chunks, so gathering starts as soon as
the first chunk of G is in DRAM.

Sharding: queries (t, s) are split 8x8192 across cores (data parallel);
obs / mu / alpha / beta are replicated.  No collectives needed.
"""

import os
import sys

import numpy as np

_REPO_CANDIDATES = ("/opt/trn_rl_repo", os.path.expanduser("~/.axon_site/_ro/trn_rl_repo"))
for _p in _REPO_CANDIDATES:
    if os.path.isdir(_p) and _p not in sys.path:
        sys.path.append(_p)

import concourse.bass as bass
import concourse.tile as tile
from concourse import bacc, mybir
from concourse.bass_utils import run_bass_kernel_spmd

# Problem constants (hardcoded per spec).
N_TIME = 4096
N_SPACE = 256
BATCH = 65536
N_CORES = 8
LAM_MIN = 1e-5

P = 128               # partitions / time-block size
J = N_TIME // P       # 32 time blocks
PER_CORE = BATCH // N_CORES   # 8192 queries per core
CH = 512              # matmul N-chunk (one PSUM bank)
NCH = (J * N_SPACE) // CH     # 16 chunks over the (j, s) flat axis

# Gather slot layout: 17 stages of 4 columns (128 queries each).  Stage c
# (c < 16) only holds queries with t < 256*(c+1), i.e. covered by G store
# chunks 0..c, so its gathers can fire right after chunk c is stored.
# Stage 16 holds the spill (any t); 8704 slots for 8192 queries.
NSTAGE = 17
SCOLS = 4
FQ = NSTAGE * SCOLS   # 68 query slot columns per partition
NSLOT = P * FQ        # 8704 slots per core

f32 = mybir.dt.float32
bf16 = mybir.dt.bfloat16
i32 = mybir.dt.int32
Alu = mybir.AluOpType
Act = mybir.ActivationFunctionType


def build_nc():
    nc = bacc.Bacc("TRN2", target_bir_lowering=False, debug=False,
                   dynamic_dma_scratch_size=65536)

    gidx_h = nc.dram_tensor("gidx", [NSLOT], i32, kind="ExternalInput")
    obs_h = nc.dram_tensor("obs", [N_TIME, N_SPACE], i32, kind="ExternalInput")
    mu_h = nc.dram_tensor("mu", [N_SPACE], f32, kind="ExternalInput")
    alpha_h = nc.dram_tensor("alpha", [N_SPACE, N_SPACE], f32, kind="ExternalInput")
    beta_h = nc.dram_tensor("beta", [1], f32, kind="ExternalInput")
    g_h = nc.dram_tensor("gtab", [N_TIME * N_SPACE + 2], f32, kind="Internal")
    out_h = nc.dram_tensor("out", [NSLOT], f32, kind="ExternalOutput")

    from contextlib import ExitStack

    with tile.TileContext(nc) as tc, ExitStack() as ctx:
        sb = ctx.enter_context(tc.tile_pool(name="sb", bufs=1))
        ps = ctx.enter_context(tc.tile_pool(name="ps", bufs=4, space="PSUM"))
        psr = ctx.enter_context(tc.tile_pool(name="psr", bufs=2, space="PSUM"))
        ps1 = ctx.enter_context(tc.tile_pool(name="ps1", bufs=1, space="PSUM"))
        sb2 = ctx.enter_context(tc.tile_pool(name="sb2", bufs=4))

        # ---- input loads -------------------------------------------------
        obs_view = obs_h.ap().rearrange("(j p) s -> p j s", p=P)
        obs_i = sb.tile([P, J, N_SPACE], i32)
        for q in range(8):
            nc.sync.dma_start(obs_i[:, 4 * q:4 * q + 4, :],
                              obs_view[:, 4 * q:4 * q + 4, :])

        beta_bc = sb.tile([P, 1], f32)
        nc.scalar.dma_start(beta_bc[:], bass.AP(beta_h, 0, [[0, P], [1, 1]]))

        adiag = sb.tile([1, N_SPACE], f32)
        nc.scalar.dma_start(adiag[:], bass.AP(alpha_h, 0, [[0, 1], [N_SPACE + 1, N_SPACE]]))

        rhs2 = sb.tile([2, J * N_SPACE], bf16)  # row0 = carry C flat, row1 = mu tiled
        mu_f = sb.tile([1, N_SPACE], f32)
        nc.scalar.dma_start(mu_f[:], bass.AP(mu_h, 0, [[0, 1], [1, N_SPACE]]))
        mu_b = sb.tile([1, N_SPACE], bf16)
        nc.vector.tensor_copy(mu_b[:], mu_f[:])
        nc.scalar.dma_start(
            rhs2[1:2, :].rearrange("o (j s) -> o j s", s=N_SPACE),
            mu_b[:].unsqueeze(1).broadcast_to((1, J, N_SPACE)))

        idx = sb.tile([P, FQ], i32)
        nc.scalar.dma_start(idx[:], bass.AP(gidx_h, 0, [[FQ, P], [1, FQ]]))

        # ---- runtime constants from beta --------------------------------
        negb = sb.tile([P, 1], f32)
        nc.vector.tensor_scalar(out=negb[:], in0=beta_bc[:], scalar1=-1.0,
                                scalar2=None, op0=Alu.mult)
        negb128 = sb.tile([P, 1], f32)
        nc.vector.tensor_scalar(out=negb128[:], in0=negb[:], scalar1=128.0,
                                scalar2=None, op0=Alu.mult)

        # LdT[tp, m] = exp(-b (m - tp)) for tp < m else 0   (within-block decay)
        xd = sb.tile([P, P], i32)
        nc.gpsimd.iota(xd[:], [[1, P]], base=0, channel_multiplier=-1)   # f - p
        lda = sb.tile([P, P], f32)
        nc.vector.tensor_scalar(out=lda[:], in0=xd[:], scalar1=negb[:],
                                scalar2=None, op0=Alu.mult)
        ldb = sb.tile([P, P], f32)
        nc.vector.tensor_scalar(out=ldb[:], in0=xd[:], scalar1=1000.0,
                                scalar2=-1000.0, op0=Alu.mult, op1=Alu.add)
        ldm = sb.tile([P, P], f32)
        nc.vector.tensor_tensor(out=ldm[:], in0=lda[:], in1=ldb[:], op=Alu.min)
        ldt = sb.tile([P, P], f32)
        nc.scalar.activation(ldt[:], ldm[:], Act.Exp)
        ldtb = sb.tile([P, P], bf16)
        nc.vector.tensor_copy(ldtb[:], ldt[:])

        # v[tp] = exp(-b (128 - tp))  (end-of-block carry weights)
        xv = sb.tile([P, 1], i32)
        nc.gpsimd.iota(xv[:], [[0, 1]], base=P, channel_multiplier=-1)   # 128 - p
        vm = sb.tile([P, 1], f32)
        nc.vector.tensor_scalar(out=vm[:], in0=xv[:], scalar1=negb[:],
                                scalar2=None, op0=Alu.mult)
        vv = sb.tile([P, 1], f32)
        nc.scalar.activation(vv[:], vm[:], Act.Exp)
        vvb = sb.tile([P, 1], bf16)
        nc.vector.tensor_copy(vvb[:], vv[:])

        # LcT[k, j] = exp(-128 b (j - 1 - k)) for k <= j-1 else 0  (carry matrix)
        xc = sb.tile([J, J], i32)
        nc.gpsimd.iota(xc[:], [[1, J]], base=-1, channel_multiplier=-1)  # f - 1 - p
        lca = sb.tile([J, J], f32)
        nc.vector.tensor_scalar(out=lca[:], in0=xc[:], scalar1=negb128[:J, :],
                                scalar2=None, op0=Alu.mult)
        lcb = sb.tile([J, J], f32)
        nc.vector.tensor_scalar(out=lcb[:], in0=xc[:], scalar1=1000.0,
                                scalar2=None, op0=Alu.mult)
        lcm = sb.tile([J, J], f32)
        nc.vector.tensor_tensor(out=lcm[:], in0=lca[:], in1=lcb[:], op=Alu.min)
        lct = sb.tile([J, J], f32)
        nc.scalar.activation(lct[:], lcm[:], Act.Exp)

        # u2: row0 = u_i = exp(-b i), row1 = ones (mu term).
        # scale vector [-b; 0] makes exp produce both rows at once.
        negb01 = sb.tile([2, 1], f32)
        nc.vector.memset(negb01[:], 0.0)
        nc.vector.tensor_copy(negb01[0:1, :], negb[0:1, :])
        xu = sb.tile([2, P], i32)
        nc.gpsimd.iota(xu[:], [[1, P]], base=0, channel_multiplier=0)    # f
        um = sb.tile([2, P], f32)
        nc.vector.tensor_scalar(out=um[:], in0=xu[:], scalar1=negb01[:],
                                scalar2=None, op0=Alu.mult)
        u2 = sb.tile([2, P], f32)
        nc.scalar.activation(u2[:], um[:], Act.Exp)
        u2b = sb.tile([2, P], bf16)
        nc.vector.tensor_copy(u2b[:], u2[:])

        # asb[s] = b * alpha[s, s], broadcast to all 128 partitions via PE
        asb_row = sb.tile([1, N_SPACE], f32)
        nc.vector.tensor_scalar(out=asb_row[:], in0=adiag[:],
                                scalar1=beta_bc[:1, :], scalar2=None, op0=Alu.mult)
        ones1 = sb.tile([1, P], f32)
        nc.vector.memset(ones1[:], 1.0)
        asb_ps = ps1.tile([P, N_SPACE], f32)
        nc.tensor.matmul(asb_ps[:], lhsT=ones1[:], rhs=asb_row[:], start=True, stop=True)
        asb_bc = sb.tile([P, N_SPACE], f32)
        nc.vector.tensor_copy(asb_bc[:], asb_ps[:])

        # obs_f[tp, j, s] = obs * asb[s]   (convert + scale, 4 chunked DVE passes)
        obs_f = sb.tile([P, J * N_SPACE], bf16)
        obs_ff = obs_f[:]                # [P, 8192] flat view
        obs_f3 = obs_f[:].rearrange("p (j s) -> p j s", s=N_SPACE)
        for q in range(4):
            nc.vector.tensor_tensor(
                out=obs_f3[:, 8 * q:8 * q + 8, :],
                in0=obs_i[:, 8 * q:8 * q + 8, :],
                in1=asb_bc[:].unsqueeze(1).broadcast_to((P, 8, N_SPACE)),
                op=Alu.mult,
            )

        # ---- fused build + per-chunk gather pipeline --------------------
        r_flat = sb.tile([1, J * N_SPACE], f32)
        r32 = sb.tile([J, N_SPACE], f32)
        rhs2_j = rhs2[0:1, :].rearrange("o (j s) -> o j s", s=N_SPACE)
        g_store = bass.AP(g_h, 0, [[N_SPACE, P], [P * N_SPACE, J], [1, N_SPACE]])

        gath = sb.tile([P, 2 * FQ], f32)
        views = [bass.AP(g_h, 0, [[1, min(256 * (c + 1), N_TIME) * N_SPACE], [1, 1]])
                 for c in range(NSTAGE - 1)]
        views.append(bass.AP(g_h, 0, [[1, N_TIME * N_SPACE + 2], [1, 1]]))
        zpad = sb.tile([1, 2], f32)
        nc.vector.memset(zpad[:], 0.0)
        nc.sync.dma_start(bass.AP(g_h, N_TIME * N_SPACE, [[1, 1], [1, 2]]), zpad[:])

        def fire_stage(st):
            for f in range(SCOLS * st, SCOLS * (st + 1)):
                nc.gpsimd.indirect_dma_start(
                    out=gath[:, 2 * f:2 * f + 2],
                    out_offset=None,
                    in_=views[st],
                    in_offset=bass.IndirectOffsetOnAxis(ap=idx[:, f:f + 1],
                                                        axis=0),
                )
            lam = sb2.tile([P, SCOLS], f32, tag="lam")
            nc.vector.tensor_scalar(
                out=lam[:].rearrange("p (f o) -> p f o", o=1),
                in0=gath[:].rearrange("p (f o) -> p f o", o=2)[
                    :, SCOLS * st:SCOLS * (st + 1), 0:1],
                scalar1=float(LAM_MIN), scalar2=None, op0=Alu.max)
            nc.scalar.dma_start(
                bass.AP(out_h, SCOLS * st, [[FQ, P], [1, SCOLS]]), lam[:])

        def emit_r(c):
            r_ps = psr.tile([1, CH], f32)
            nc.tensor.matmul(r_ps[:], lhsT=vvb[:],
                             rhs=obs_ff[:, c * CH:(c + 1) * CH],
                             start=True, stop=True)
            nc.scalar.activation(r_flat[:, c * CH:(c + 1) * CH], r_ps[:],
                                 Act.Copy)

        def emit_gchunk(c):
            pch = ps.tile([P, CH], f32)
            nc.tensor.matmul(pch[:], lhsT=ldtb[:],
                             rhs=obs_ff[:, c * CH:(c + 1) * CH],
                             start=True, stop=True)
            nc.tensor.matmul(pch[:], lhsT=u2b[:],
                             rhs=rhs2[:, c * CH:(c + 1) * CH],
                             start=False, stop=True, skip_group_check=True)
            gch = sb2.tile([P, CH], f32, tag="gch")
            if c % 2 == 0:
                nc.vector.tensor_copy(gch[:], pch[:])
            else:
                nc.scalar.activation(gch[:], pch[:], Act.Copy)
            jj = c * CH // N_SPACE
            eng = nc.sync if c % 2 == 0 else nc.scalar
            eng.dma_start(g_store[:, jj:jj + CH // N_SPACE, :], gch[:])
            # chunk c covers t in [256c, 256c+256): stage c unlocked.
            fire_stage(c)

        for k in range(4):
            if k == 0:
                # fast path for chunk 0: its carry rows are C[0] = 0 and
                # C[1] = z[0] (= r of chunk 0, first half), so it can build,
                # store, and start gathering before the quarter carry chain.
                emit_r(0)
                nc.vector.memset(rhs2_j[:, 0:1, :], 0.0)
                nc.vector.tensor_copy(rhs2_j[:, 1:2, :],
                                      r_flat[:, 0:N_SPACE].unsqueeze(1))
                emit_gchunk(0)
                for c in range(1, 4):
                    emit_r(c)
            else:
                for c in range(4 * k, 4 * k + 4):
                    emit_r(c)
            nc.sync.dma_start(r32[8 * k:8 * k + 8, :],
                              r_flat[:, 2048 * k:2048 * (k + 1)])
            c_ps = ps1.tile([8, N_SPACE], f32, tag="cps")
            nc.tensor.matmul(c_ps[:], lhsT=lct[0:8 * (k + 1), 8 * k:8 * (k + 1)],
                             rhs=r32[0:8 * (k + 1), :], start=True, stop=True)
            c32 = sb2.tile([8, N_SPACE], bf16, tag="c32")
            nc.vector.tensor_copy(c32[:], c_ps[:])
            nc.sync.dma_start(rhs2_j[:, 8 * k:8 * k + 8, :], c32[:])

            for c in range(4 * k + (1 if k == 0 else 0), 4 * k + 4):
                emit_gchunk(c)
        fire_stage(16)

    nc.compile()
    return nc


_NC_CACHE = None


def _get_nc():
    global _NC_CACHE
    if _NC_CACHE is None:
        _NC_CACHE = build_nc()
    return _NC_CACHE


def _route_queries(tc_, sc_):
    """Route one core's queries into gather-stage slots.

    Stage c < 16 (capacity 512 = 4 columns x 128 partitions) may only hold
    queries with t < 256*(c+1); stages 16-17 hold anything.  Query q of
    stage c lands in device slot (p, col) = (q % 128, 4c + q // 128).

    Returns (gidx [NSLOT] int32 flat t*256+s per slot, (dev_pos, orig_pos))
    with out[orig_pos] = dev_out[dev_pos] on the [NSLOT] flat device
    output (flat = p * FQ + col).
    """
    n = tc_.shape[0]
    order = np.argsort(tc_, kind="stable")
    ts = tc_[order]
    flat_all = tc_.astype(np.int64) * N_SPACE + sc_.astype(np.int64)

    gidx = np.zeros((P, FQ), np.int32)
    dev_parts, orig_parts = [], []
    lo = 0
    cap = P * SCOLS
    for c in range(NSTAGE):
        bound = min(256 * (c + 1), N_TIME)
        hi = np.searchsorted(ts, bound, side="left")
        take = min(cap, hi - lo) if c < NSTAGE - 1 else (n - lo)
        if take > cap:
            raise RuntimeError("query t-distribution infeasible for stages")
        sel = order[lo:lo + take]
        q = np.arange(take)
        p = q % P
        col = SCOLS * c + q // P
        gidx[p, col] = flat_all[sel]
        dev_parts.append(p * FQ + col)
        orig_parts.append(sel)
        lo += take
    dev_pos = np.concatenate(dev_parts)
    orig_pos = np.concatenate(orig_parts)
    return gidx.reshape(-1), (dev_pos, orig_pos)


def _make_in_maps(t, s, obs, mu, alpha, beta):
    in_maps, perms = [], []
    for c in range(N_CORES):
        sl = slice(c * PER_CORE, (c + 1) * PER_CORE)
        gidx, perm = _route_queries(t[sl], s[sl])
        perms.append(perm)
        in_maps.append({
            "gidx": gidx,
            "obs": obs, "mu": mu, "alpha": alpha, "beta": beta,
        })
    return in_maps, perms


def kernel(t, s, obs, mu, alpha, beta, **_unused):
    t = np.ascontiguousarray(np.asarray(t, dtype=np.int32))
    s = np.ascontiguousarray(np.asarray(s, dtype=np.int32))
    obs = np.ascontiguousarray(np.asarray(obs, dtype=np.int32))
    mu = np.ascontiguousarray(np.asarray(mu, dtype=np.float32))
    alpha = np.ascontiguousarray(np.asarray(alpha, dtype=np.float32))
    beta = np.ascontiguousarray(np.asarray(beta, dtype=np.float32))

    nc = _get_nc()
    in_maps, perms = _make_in_maps(t, s, obs, mu, alpha, beta)
    res = run_bass_kernel_spmd(nc, in_maps, core_ids=list(range(N_CORES)))
    outs = []
    for c in range(N_CORES):
        dev = res.results[c]["out"]          # [NSLOT]
        o = np.empty(PER_CORE, np.float32)
        o[perms[c][1]] = dev[perms[c][0]]
        outs.append(o)
    return np.concatenate(outs).astype(np.float32)


if __name__ == "__main__":
    # quick self-check against a numpy re-implementation on random data
    rng = np.random.default_rng(0)
    t = rng.integers(0, N_TIME, BATCH).astype(np.int32)
    s = rng.integers(0, N_SPACE, BATCH).astype(np.int32)
    obs = rng.integers(0, 10, (N_TIME, N_SPACE)).astype(np.int32)
    mu = rng.random(N_SPACE, dtype=np.float32)
    alpha = rng.random((N_SPACE, N_SPACE), dtype=np.float32)
    beta = (rng.random(1, dtype=np.float32) + 0.1).astype(np.float32)

    got = kernel(t=t, s=s, obs=obs, mu=mu, alpha=alpha, beta=beta)

    b = float(beta[0])
    e = np.exp(-b)
    F = np.zeros((N_TIME, N_SPACE), np.float64)
    for tt in range(1, N_TIME):
        F[tt] = e * (F[tt - 1] + obs[tt - 1])
    G = np.clip(mu[None, :] + np.diag(alpha)[None, :] * b * F, LAM_MIN, None)
    want = G[t, s].astype(np.float32)
    err = np.abs(got - want) / np.maximum(np.abs(want), 1e-6)
    print("max rel err:", err.max(), "mean:", err.mean())
